# revision 8
# baseline (speedup 1.0000x reference)
"""Trainium2 Bass kernel for a GPT-style decoder block (B=2, T=2048, d=1024,
16 heads, FFN 4096), distributed over 8 NeuronCores.

Sharding: DP2 (batch) x TP4 (4 heads + proj-row split per core). The single
collective is a per-token-quarter ReduceScatter of the attention projection
partials over each 4-core group; after it, every core owns its token strips
and runs LN2+FFN (full hidden dim) on just those, writing its 512-token
output slice.

v4 vs v3 baseline:
- x staged pre-transposed (xT) from host; LN1 computed in d-major layout via
  ones-matmul token stats + broadcast rows (eliminates 128 PE transposes).
- LN1 gamma/beta folded into W_qkv / qkv biases; LN2 gamma/beta folded into
  W_fc / b_fc (host-side constant prep).
- Scores per head-pair packed into one [128,2,512] PSUM tile via row-tiled
  concurrent matmuls; one exp per pair-block; col-restricted exp/scores/PV
  on causal-diagonal blocks.
- qkv bias adds on DVE; rstd via gpsimd pow -> ACT does only Exp + Gelu
  (no activation-table thrash).
- Per-quarter residual+LN2 inline right after each ReduceScatter lands
  (overlaps attention).
- Front phase software-pipelined: span s+1 token stats run on PE before
  span s QKV so the LN broadcast chain hides under matmuls.

Self-contained: hardcodes all shapes; no sibling imports.
"""
import numpy as np

import concourse.bacc as bacc
import concourse.mybir as mybir
import concourse.tile as tile
from concourse.bass_utils import run_bass_kernel_spmd
from concourse.masks import make_identity

F32 = mybir.dt.float32
F16 = mybir.dt.float16
AF = mybir.ActivationFunctionType
OP = mybir.AluOpType

P = 128
T = 2048          # tokens per batch element
D = 1024          # embed dim
NT = T // P       # 16 token tiles
DC = D // P       # 8 d-chunks
FH = 4            # heads per core
DH = 64           # head dim
FQ = 256          # q (=k=v) features per core
HID = 4096        # full FFN hidden
TS = 512          # token slice per core
NQ = 4            # token quarters
SPAN = 512        # attention query span
NSPAN = T // SPAN
EPS = 1e-5
GROUPS = [[0, 1, 2, 3], [4, 5, 6, 7]]
SKEW = 3          # score pair-blocks run this far ahead of the PV chain


def build_nc():
    nc = bacc.Bacc(None, target_bir_lowering=False)

    # ---- external I/O ----
    xt_d = nc.dram_tensor("xt", [DC, P, T], F16, kind="ExternalInput")
    xs_d = nc.dram_tensor("xs", [TS, D], F16, kind="ExternalInput")
    wq_d = nc.dram_tensor("wq", [D, FQ], F16, kind="ExternalInput")
    wk_d = nc.dram_tensor("wk", [D, FQ], F16, kind="ExternalInput")
    wv_d = nc.dram_tensor("wv", [D, FQ], F16, kind="ExternalInput")
    bqk_d = nc.dram_tensor("bqk", [P, 4], F32, kind="ExternalInput")
    bvb_d = nc.dram_tensor("bvb", [P, FQ], F32, kind="ExternalInput")
    wp_d = nc.dram_tensor("wp", [FQ, D], F16, kind="ExternalInput")
    wfc_d = nc.dram_tensor("wfc", [HID // P, P, DC, P], F16,
                           kind="ExternalInput")
    bfc_d = nc.dram_tensor("bfc", [P, HID // P], F32, kind="ExternalInput")
    wfc2_d = nc.dram_tensor("wfc2", [HID, D], F16, kind="ExternalInput")
    bfc2b_d = nc.dram_tensor("bfc2b", [P, D], F32, kind="ExternalInput")
    out_d = nc.dram_tensor("out", [TS, D], F32, kind="ExternalOutput")

    rs_in = [nc.dram_tensor(f"rs_in{q}", [SPAN, D], F16) for q in range(NQ)]
    rs_out = [nc.dram_tensor(f"rs_out{q}", [P, D], F16) for q in range(NQ)]

    with tile.TileContext(nc) as tc:
        cst_cm = tc.tile_pool(name="cst", bufs=1)
        cst = cst_cm.__enter__()
        big_cm = tc.tile_pool(name="big", bufs=1)
        big = big_cm.__enter__()

        # ---- first xT tiles before anything else hits the DMA queues ----
        pxt_cm = tc.tile_pool(name="p_xt", bufs=2)
        p_xt = pxt_cm.__enter__()
        xts = []
        xt0 = p_xt.tile([P, DC, SPAN], F16, tag="xt", name="xt0")
        nc.sync.dma_start(xt0[:], xt_d[:, :, 0:SPAN].rearrange("c p t -> p c t"))
        xts.append(xt0)

        # ---- constants / small params ----
        identf = cst.tile([P, P], F32)
        make_identity(nc, identf[:])
        ident = cst.tile([P, P], F16)
        nc.vector.tensor_copy(ident[:], identf[:])
        ones4 = cst.tile([P, FH, 1], F16)
        with nc.allow_low_precision("exact value 1.0"):
            nc.gpsimd.memset(ones4[:], 1.0)
        ones1 = cst.tile([P, 1], F16)
        with nc.allow_low_precision("exact value 1.0"):
            nc.gpsimd.memset(ones1[:], 1.0)
        epsb = cst.tile([P, 1], F32)
        nc.gpsimd.memset(epsb[:], EPS)

        bqkt = cst.tile([P, 4], F32)
        nc.sync.dma_start(bqkt[:], bqk_d[:])
        bfct = cst.tile([P, HID // P], F32)
        nc.sync.dma_start(bfct[:], bfc_d[:])
        bvb = cst.tile([P, FQ], F32)
        nc.sync.dma_start(bvb[:], bvb_d[:])
        bfc2b = cst.tile([P, D], F32)
        nc.sync.dma_start(bfc2b[:], bfc2b_d[:])

        # ---- resident weights ----
        wq_t = big.tile([P, DC, FQ], F16, name="wq_t")
        wk_t = big.tile([P, DC, FQ], F16, name="wk_t")
        wv_t = big.tile([P, DC, FQ], F16, name="wv_t")
        wp_t = big.tile([P, 2, D], F16, name="wp_t")
        nc.sync.dma_start(wq_t[:], wq_d[:, :].rearrange("(c p) f -> p c f", p=P))
        nc.sync.dma_start(wk_t[:], wk_d[:, :].rearrange("(c p) f -> p c f", p=P))
        nc.sync.dma_start(wv_t[:], wv_d[:, :].rearrange("(c p) f -> p c f", p=P))
        nc.sync.dma_start(wp_t[:], wp_d[:, :].rearrange("(c p) f -> p c f", p=P))
        # fc1 weight, fully preloaded (DMA issued at attention start);
        # lives in `big` so it survives until the FFN phase
        wfc_sb = big.tile([P, HID // P, DC, P], F16, name="wfc_sb")

        # ---- big tag-shared tiles (front/attention lives) ----
        h1T = big.tile([P, DC, T], F16, tag="A", name="h1T")
        qT = big.tile([P, 2, T], F16, tag="B1", name="qT")
        kT = big.tile([P, 2, T], F16, tag="B2", name="kT")
        vhat = big.tile([P, NT, FH * (DH + 1)], F16, tag="B3", name="vhat")
        attT = big.tile([P, 2, T], F16, tag="B4", name="attT")

        # ============ Front: LN1 (d-major) + QKV, software-pipelined =======
        qk_meta = [(wq_t, 0, qT, 0), (wq_t, 1, qT, 1),
                   (wk_t, 0, kT, 0), (wk_t, 1, kT, 1)]
        with tc.tile_pool(name="p_sq", bufs=3) as p_sq, \
             tc.tile_pool(name="p_ln", bufs=2) as p_ln, \
             tc.tile_pool(name="ps_st", bufs=2, space="PSUM") as ps_st, \
             tc.tile_pool(name="ps_qk", bufs=2, space="PSUM") as ps_qk, \
             tc.tile_pool(name="ps_v", bufs=2, space="PSUM") as ps_v:

            def stats(s):
                """Token stats for span s -> normalized xhat_T into h1T."""
                if s == 0:
                    xt = xts[0]
                else:
                    xt = p_xt.tile([P, DC, SPAN], F16, tag="xt",
                                   name=f"xt{s}")
                    nc.sync.dma_start(
                        xt[:],
                        xt_d[:, :, s * SPAN:(s + 1) * SPAN]
                        .rearrange("c p t -> p c t"))
                    xts.append(xt)
                sums = ps_st.tile([1, SPAN], F32, tag="sums", name=f"sums{s}")
                sqs = ps_st.tile([33, SPAN], F32, tag="sqs", name=f"sqs{s}")
                for c in range(DC):
                    xsq = p_sq.tile([P, SPAN], F16, tag="xsq",
                                    name=f"xsq{s}_{c}")
                    with nc.allow_low_precision("fp16 activations"):
                        nc.vector.tensor_tensor(xsq[:], xt[:, c, :],
                                                xt[:, c, :], OP.mult)
                    nc.tensor.matmul(sums[0:1, :], ones1[:], xt[:, c, :],
                                     start=(c == 0), stop=(c == DC - 1))
                    nc.tensor.matmul(sqs[32:33, :], ones1[:], xsq[:],
                                     start=(c == 0), stop=(c == DC - 1))
                # mu = sum/D ; var = sumsq/D - mu^2 ; rstd = (var+eps)^-1/2
                st = p_ln.tile([1, 5, SPAN], F32, tag="st", bufs=1,
                               name=f"st{s}")
                mu, var, sd, rstd, mrs = (st[:, i, :] for i in range(5))
                nc.vector.tensor_scalar(mu, sums[0:1, :], 1.0 / D, None,
                                        OP.mult)
                nc.vector.scalar_tensor_tensor(var, mu, -1.0, mu,
                                               op0=OP.mult, op1=OP.mult)
                nc.vector.scalar_tensor_tensor(var, sqs[32:33, :], 1.0 / D,
                                               var, op0=OP.mult, op1=OP.add)
                nc.scalar.activation(sd, var, AF.Sqrt, bias=epsb[0:1, :])
                nc.vector.reciprocal(rstd, sd)
                nc.vector.tensor_tensor(mrs, mu, rstd, OP.mult)
                rows = p_ln.tile([1, 2, SPAN], F16, tag="rows", bufs=1,
                                 name=f"rows{s}")
                with nc.allow_low_precision("fp16 activations"):
                    nc.vector.tensor_copy(rows[:, 0, :], rstd)
                    nc.vector.tensor_copy(rows[:, 1, :], mrs)
                bc = p_ln.tile([P, 2, SPAN], F16, tag="bc", name=f"bc{s}")
                nc.gpsimd.partition_broadcast(bc[:, 0, :], rows[:, 0, :])
                nc.gpsimd.partition_broadcast(bc[:, 1, :], rows[:, 1, :])
                # xhat_T = xT * R - M
                with nc.allow_low_precision("fp16 activations"):
                    for c in range(DC):
                        nc.vector.tensor_tensor(
                            h1T[:, c, s * SPAN:(s + 1) * SPAN],
                            xt[:, c, :], bc[:, 0, :], OP.mult)
                        nc.vector.tensor_tensor(
                            h1T[:, c, s * SPAN:(s + 1) * SPAN],
                            h1T[:, c, s * SPAN:(s + 1) * SPAN],
                            bc[:, 1, :], OP.subtract)

            def qkv(s):
                for fb in range(4):
                    wsrc, half, dest, dhalf = qk_meta[fb]
                    pq = ps_qk.tile([P, SPAN], F32, tag="qk",
                                    name=f"qk{fb}_{s}")
                    for kc in range(DC):
                        nc.tensor.matmul(
                            pq[:], wsrc[:, kc, half * P:(half + 1) * P],
                            h1T[:, kc, s * SPAN:(s + 1) * SPAN],
                            start=(kc == 0), stop=(kc == DC - 1))
                    with nc.allow_low_precision("fp16 activations"):
                        nc.vector.tensor_scalar(
                            dest[:, dhalf, s * SPAN:(s + 1) * SPAN], pq[:],
                            bqkt[:, fb:fb + 1], None, OP.add)
                for m in range(4 * s, 4 * s + 4):
                    pv = ps_v.tile([P, FQ], F32, tag="v", name=f"v{m}")
                    for kc in range(DC):
                        nc.tensor.matmul(
                            pv[:], h1T[:, kc, m * P:(m + 1) * P],
                            wv_t[:, kc, :],
                            start=(kc == 0), stop=(kc == DC - 1))
                    vdst = vhat[:, m, :].rearrange("p (h x) -> p h x",
                                                   x=DH + 1)
                    with nc.allow_low_precision("fp16 activations"):
                        nc.vector.tensor_tensor(
                            vdst[:, :, 0:DH],
                            pv[:].rearrange("p (h x) -> p h x", x=DH),
                            bvb[:].rearrange("p (h x) -> p h x", x=DH),
                            OP.add)
                        nc.vector.tensor_copy(vdst[:, :, DH:DH + 1], ones4[:])

            stats(0)
            for s in range(NSPAN):
                if s + 1 < NSPAN:
                    stats(s + 1)
                qkv(s)
        pxt_cm.__exit__(None, None, None)

        # prefetch fc1 weights into SBUF while attention computes
        for pq4 in range(4):
            m0 = pq4 * (HID // P // 4)
            m1 = (pq4 + 1) * (HID // P // 4)
            nc.sync.dma_start(
                wfc_sb[:, m0:m1, :, :],
                wfc_d[m0:m1].rearrange("m p c f -> p m c f"))

        # ---- phase-5 persistent tiles ----
        x2 = big.tile([P, NQ, D], F16, tag="B1x", name="x2")
        h2T = big.tile([P, DC, TS], F16, tag="B2x", name="h2T")
        mvq = big.tile([P, NQ, 2], F32, name="mvq")

        # per-quarter residual + LN2 stats, DVE-only (runs during attention)
        def p5_early(q, p_rs, p_l2):
            rst = p_rs.tile([P, D], F16, tag="rst", name=f"rst{q}")
            nc.sync.dma_start(rst[:], rs_out[q][:])
            xsq_ = p_rs.tile([P, D], F16, tag="xsl", name=f"xsl{q}")
            nc.sync.dma_start(xsq_[:], xs_d[q * P:(q + 1) * P, :])
            with nc.allow_low_precision("fp16 residual"):
                nc.vector.tensor_tensor(x2[:, q, :], rst[:], xsq_[:], OP.add)
            bn6 = p_l2.tile([P, 2, 6], F32, tag="bn6", name=f"bn6_{q}")
            for a in range(2):
                nc.vector.bn_stats(bn6[:, a, :],
                                   x2[:, q, a * 512:(a + 1) * 512])
            nc.vector.bn_aggr(mvq[:, q, :], bn6[:])

        # normalize + transpose (post-attention; Sqrt shares no exp table)
        def p5_late(q, p_l2, ps_t2):
            sd = p_l2.tile([P, 1], F32, tag="sd2", name=f"sd2_{q}")
            nc.scalar.activation(sd[:], mvq[:, q, 1:2], AF.Sqrt,
                                 bias=epsb[:])
            rstd = p_l2.tile([P, 1], F32, tag="rstd2", name=f"rstd2_{q}")
            nc.vector.reciprocal(rstd[:], sd[:])
            xh2 = p_l2.tile([P, D], F16, tag="xh2", name=f"xh2_{q}")
            with nc.allow_low_precision("fp16 activations"):
                nc.vector.tensor_scalar(
                    xh2[:], x2[:, q, :], mvq[:, q, 0:1], rstd[:],
                    OP.subtract, OP.mult)
            for j in range(DC):
                pt = ps_t2.tile([P, P], F16, tag="t2", name=f"t2_{q}_{j}")
                nc.tensor.transpose(pt[:], xh2[:, j * P:(j + 1) * P],
                                    ident[:])
                with nc.allow_low_precision("fp16 activations"):
                    nc.vector.tensor_copy(h2T[:, j, q * P:(q + 1) * P], pt[:])

        # ================= Attention + proj + ReduceScatter =================
        with tc.tile_pool(name="p_e", bufs=1) as p_e, \
             tc.tile_pool(name="p_pr", bufs=3) as p_pr, \
             tc.tile_pool(name="p_rs", bufs=2) as p_rs, \
             tc.tile_pool(name="p_l2", bufs=2) as p_l2, \
             tc.tile_pool(name="ps_s", bufs=2, space="PSUM") as ps_s, \
             tc.tile_pool(name="ps_pv", bufs=1, space="PSUM") as ps_pv, \
             tc.tile_pool(name="ps_t2", bufs=2, space="PSUM") as ps_t2:
            for s in range(NSPAN):
                nkb = (s + 1) * (SPAN // P)
                for hp in range(2):
                    ppvs = [ps_pv.tile([DH + 1, SPAN], F32, tag=f"pv{i}",
                                       name=f"pv{hp * 2 + i}_{s}")
                            for i in range(2)]
                    es = {}

                    def scores(kb, s=s, hp=hp, es=es):
                        j = kb - s * (SPAN // P)
                        c0 = max(0, j) * P   # first valid query col
                        pst = ps_s.tile([P, 2, SPAN], F32, tag="sT",
                                        name=f"sT{hp}_{s}_{kb}")
                        for hh in range(2):
                            nc.tensor.matmul(
                                pst[:, hh, c0:SPAN],
                                kT[hh * DH:(hh + 1) * DH, hp,
                                   kb * P:(kb + 1) * P],
                                qT[hh * DH:(hh + 1) * DH, hp,
                                   s * SPAN + c0:(s + 1) * SPAN])
                        e = p_e.tile([P, 2, SPAN], F16, tag="e",
                                     bufs=SKEW + 2, name=f"e{hp}_{s}_{kb}")
                        with nc.allow_low_precision("fp16 activations"):
                            nc.scalar.activation(e[:, :, c0:SPAN],
                                                 pst[:, :, c0:SPAN],
                                                 AF.Exp, scale=0.125)
                        if j >= 0:
                            # zero the causal triangle of the diagonal block
                            nc.gpsimd.affine_select(
                                out=e[:, :, c0:c0 + P],
                                in_=e[:, :, c0:c0 + P],
                                compare_op=OP.is_ge, fill=0.0, base=0,
                                channel_multiplier=-1,
                                pattern=[[0, 2], [1, P]])
                        es[kb] = (e, c0)

                    def pv_acc(kb, s=s, hp=hp, ppvs=ppvs, nkb=nkb, es=es):
                        e, c0 = es.pop(kb)
                        for hh in range(2):
                            h = hp * 2 + hh
                            nc.tensor.matmul(
                                ppvs[hh][:, c0:SPAN],
                                vhat[:, kb, h * (DH + 1):(h + 1) * (DH + 1)],
                                e[:, hh, c0:SPAN], start=(kb == 0),
                                stop=(kb == nkb - 1))

                    for i in range(nkb + SKEW):
                        if i < nkb:
                            scores(i)
                        if i >= SKEW:
                            pv_acc(i - SKEW)
                    # normalize: att = pv / den (recip of den row, bcast)
                    for hh in range(2):
                        ppv = ppvs[hh]
                        den = p_e.tile([1, SPAN], F32, tag="den", bufs=2,
                                       name=f"den{hp}_{hh}_{s}")
                        nc.vector.reciprocal(den[:], ppv[DH:DH + 1, :])
                        rbs = p_e.tile([DH, SPAN], F32, tag="rbs", bufs=2,
                                       name=f"rbs{hp}_{hh}_{s}")
                        nc.gpsimd.partition_broadcast(rbs[:], den[:],
                                                      channels=DH)
                        with nc.allow_low_precision("fp16 activations"):
                            nc.vector.tensor_tensor(
                                attT[hh * DH:(hh + 1) * DH, hp,
                                     s * SPAN:(s + 1) * SPAN],
                                ppv[0:DH, :], rbs[:], OP.mult)

                # projection partial for this token quarter, then its RS
                for mtl in range(4):
                    m = s * 4 + mtl
                    pp = ps_s.tile([P, 2, SPAN], F32, tag="sT",
                                   name=f"pr{m}")
                    for kc in range(2):
                        for n in range(2):
                            nc.tensor.matmul(
                                pp[:, n, :], attT[:, kc, m * P:(m + 1) * P],
                                wp_t[:, kc, n * SPAN:(n + 1) * SPAN],
                                start=(kc == 0), stop=(kc == 1))
                    pe = p_pr.tile([P, D], F16, tag="pe", name=f"pe{m}")
                    with nc.allow_low_precision("fp16 rs payload"):
                        nc.vector.tensor_copy(pe[:], pp[:])
                    nc.sync.dma_start(
                        rs_in[s][mtl * P:(mtl + 1) * P, :], pe[:])
                nc.gpsimd.collective_compute(
                    "ReduceScatter", OP.add, replica_groups=GROUPS,
                    ins=[rs_in[s][:]], outs=[rs_out[s][:]])
                # fold completed earlier quarters into LN2 stats while
                # attention continues (quarter q's RS lands during span q+1)
                if s >= 1:
                    p5_early(s - 1, p_rs, p_l2)
            p5_early(NQ - 1, p_rs, p_l2)
            for q in range(NQ):
                p5_late(q, p_l2, ps_t2)

        # ================= FFN =================
        # aT reuses the dead qT/kT/vhat/attT slots (4 x 8 hid-tiles)
        aTs = [big.tile([P, HID // P // 4, TS], F16, tag=f"B{i + 1}",
                        name=f"aT{i}") for i in range(4)]

        def aT(m):
            return aTs[m // 8][:, m % 8, :]

        with tc.tile_pool(name="ps_f1", bufs=4, space="PSUM") as ps_f1:
            for m in range(HID // P):
                pf = ps_f1.tile([P, TS], F32, tag="f1", name=f"f1_{m}")
                for kc in range(DC):
                    nc.tensor.matmul(pf[:], wfc_sb[:, m, kc, :], h2T[:, kc, :],
                                     start=(kc == 0), stop=(kc == DC - 1))
                with nc.allow_low_precision("fp16 activations"):
                    nc.scalar.activation(aT(m), pf[:], AF.Gelu,
                                         bias=bfct[:, m:m + 1])

        with tc.tile_pool(name="p_w2", bufs=6) as p_w2, \
             tc.tile_pool(name="p_ot", bufs=3) as p_ot, \
             tc.tile_pool(name="ps_f2", bufs=4, space="PSUM") as ps_f2:
            p4s = [ps_f2.tile([P, 2, SPAN], F32, tag="f2", name=f"f2_{mt}")
                   for mt in range(NQ)]
            for kc in range(HID // P):
                w2 = p_w2.tile([P, D], F16, tag="w2", name=f"w2_{kc}")
                nc.sync.dma_start(w2[:], wfc2_d[kc * P:(kc + 1) * P, :])
                for mt in range(NQ):
                    for n in range(2):
                        nc.tensor.matmul(
                            p4s[mt][:, n, :],
                            aT(kc)[:, mt * P:(mt + 1) * P],
                            w2[:, n * SPAN:(n + 1) * SPAN],
                            start=(kc == 0), stop=(kc == HID // P - 1))
            for mt in range(NQ):
                for n in range(2):
                    ot = p_ot.tile([P, SPAN], F32, tag="ot",
                                   name=f"ot{n}_{mt}")
                    nc.vector.tensor_tensor(
                        ot[:], p4s[mt][:, n, :],
                        x2[:, mt, n * SPAN:(n + 1) * SPAN], OP.add)
                    nc.vector.tensor_tensor(
                        ot[:], ot[:],
                        bfc2b[:, n * SPAN:(n + 1) * SPAN], OP.add)
                    nc.sync.dma_start(
                        out_d[mt * P:(mt + 1) * P, n * SPAN:(n + 1) * SPAN],
                        ot[:])

        big_cm.__exit__(None, None, None)
        cst_cm.__exit__(None, None, None)

    nc.finalize()
    return nc


def shard_inputs(inputs):
    """Full inputs -> per-core in_maps (8 cores)."""
    f = lambda a: np.ascontiguousarray(np.asarray(a, dtype=np.float32))
    h = lambda a: np.ascontiguousarray(np.asarray(a, dtype=np.float16))
    x = f(inputs["x"])
    w_attn, b_attn = f(inputs["w_attn"]), f(inputs["b_attn"])
    w_proj, b_proj = f(inputs["w_proj"]), f(inputs["b_proj"])
    ln1_g, ln1_b = f(inputs["ln1_g"]), f(inputs["ln1_b"])
    ln2_g, ln2_b = f(inputs["ln2_g"]), f(inputs["ln2_b"])
    w_fc, b_fc = f(inputs["w_fc"]), f(inputs["b_fc"])
    w_fc2, b_fc2 = f(inputs["w_fc2"]), f(inputs["b_fc2"])

    # fold LN1 gamma into W_attn rows, beta into the qkv bias
    # LN(x)@W = xhat@(g*W) + b@W  (b uses the unscaled W)
    w_attn_f = w_attn * ln1_g[:, None]
    b_attn_f = b_attn + ln1_b @ w_attn
    # fold LN2 gamma into W_fc rows, beta into b_fc
    w_fc_f = w_fc * ln2_g[:, None]
    b_fc_f = b_fc + ln2_b @ w_fc

    wfc_r = h(w_fc_f.reshape(DC, P, HID // P, P).transpose(2, 1, 0, 3))
    bfct = np.ascontiguousarray(b_fc_f.reshape(HID // P, P).T)
    bfc2b = np.ascontiguousarray(np.broadcast_to(b_fc2.reshape(1, D), (P, D)))
    wfc2_h = h(w_fc2)

    in_maps = []
    for c in range(8):
        g, tp = c // 4, c % 4
        sl = slice(tp * FQ, (tp + 1) * FQ)
        bq = b_attn_f[0 * D:1 * D][sl]
        bk = b_attn_f[1 * D:2 * D][sl]
        bv = b_attn_f[2 * D:3 * D][sl]
        bqk = np.concatenate([bq.reshape(2, P).T, bk.reshape(2, P).T], axis=1)
        bvb = np.broadcast_to(bv.reshape(1, FQ), (P, FQ))
        xs = np.concatenate(
            [x[g, q * SPAN + tp * P: q * SPAN + (tp + 1) * P]
             for q in range(NQ)], axis=0) + b_proj.reshape(1, D)
        xt = np.ascontiguousarray(x[g].T.reshape(DC, P, T))
        in_maps.append({
            "xt": h(xt),
            "xs": h(xs),
            "wq": h(w_attn_f[:, 0 * D:1 * D][:, sl]),
            "wk": h(w_attn_f[:, 1 * D:2 * D][:, sl]),
            "wv": h(w_attn_f[:, 2 * D:3 * D][:, sl]),
            "bqk": np.ascontiguousarray(bqk),
            "bvb": np.ascontiguousarray(bvb),
            "wp": h(w_proj[sl, :]),
            "wfc": wfc_r, "bfc": bfct,
            "wfc2": wfc2_h, "bfc2b": bfc2b,
        })
    return in_maps


def assemble(results):
    out = np.empty((2, T, D), dtype=np.float32)
    for c in range(8):
        g, tp = c // 4, c % 4
        r = np.asarray(results[c]["out"])
        for q in range(NQ):
            out[g, q * SPAN + tp * P: q * SPAN + (tp + 1) * P] = \
                r[q * P:(q + 1) * P]
    return out


_NC = None


def kernel(**inputs):
    global _NC
    if _NC is None:
        _NC = build_nc()
    in_maps = shard_inputs(inputs)
    res = run_bass_kernel_spmd(_NC, in_maps, list(range(8)))
    return assemble(res.results)


# revision 16
# speedup vs baseline: 1.0852x; 1.0852x over previous
"""Trainium2 Bass kernel for a GPT-style decoder block (B=2, T=2048, d=1024,
16 heads, FFN 4096), distributed over 8 NeuronCores.

Sharding: DP2 (batch) x TP4 (4 heads + proj-row split per core). The single
collective is a per-token-quarter ReduceScatter of the attention projection
partials over each 4-core group; after it, every core owns its token strips
and runs LN2+FFN (full hidden dim) on just those, writing its 512-token
output slice.

v4 vs v3 baseline:
- x staged pre-transposed (xT) from host; LN1 computed in d-major layout via
  ones-matmul token stats + broadcast rows (eliminates 128 PE transposes).
- LN1 gamma/beta folded into W_qkv / qkv biases; LN2 gamma/beta folded into
  W_fc / b_fc (host-side constant prep).
- Scores per head-pair packed into one [128,2,512] PSUM tile via row-tiled
  concurrent matmuls; one exp per pair-block; col-restricted exp/scores/PV
  on causal-diagonal blocks.
- qkv bias adds on DVE; rstd via gpsimd pow -> ACT does only Exp + Gelu
  (no activation-table thrash).
- Per-quarter residual+LN2 inline right after each ReduceScatter lands
  (overlaps attention).
- Front phase software-pipelined: span s+1 token stats run on PE before
  span s QKV so the LN broadcast chain hides under matmuls.

Self-contained: hardcodes all shapes; no sibling imports.
"""
import numpy as np

import concourse.bacc as bacc
import concourse.mybir as mybir
import concourse.tile as tile
from concourse.bass_utils import run_bass_kernel_spmd
from concourse.masks import make_identity

F32 = mybir.dt.float32
F16 = mybir.dt.float16
AF = mybir.ActivationFunctionType
OP = mybir.AluOpType

P = 128
T = 2048          # tokens per batch element
D = 1024          # embed dim
NT = T // P       # 16 token tiles
DC = D // P       # 8 d-chunks
FH = 4            # heads per core
DH = 64           # head dim
FQ = 256          # q (=k=v) features per core
HID = 4096        # full FFN hidden
TS = 512          # token slice per core
NQ = 4            # token quarters
SPAN = 512        # attention query span
NSPAN = T // SPAN
EPS = 1e-5
GROUPS = [[0, 1, 2, 3], [4, 5, 6, 7]]
SKEW = 3          # score pair-blocks run this far ahead of the PV chain


def build_nc():
    nc = bacc.Bacc(None, target_bir_lowering=False)

    # ---- external I/O ----
    xt_d = nc.dram_tensor("xt", [DC, P, T], F16, kind="ExternalInput")
    xs_d = nc.dram_tensor("xs", [TS, D], F16, kind="ExternalInput")
    wq_d = nc.dram_tensor("wq", [D, FQ], F16, kind="ExternalInput")
    wk_d = nc.dram_tensor("wk", [D, FQ], F16, kind="ExternalInput")
    wv_d = nc.dram_tensor("wv", [D, FQ], F16, kind="ExternalInput")
    bqk_d = nc.dram_tensor("bqk", [P, 4], F32, kind="ExternalInput")
    bvb_d = nc.dram_tensor("bvb", [P, FQ], F32, kind="ExternalInput")
    wp_d = nc.dram_tensor("wp", [FQ, D], F16, kind="ExternalInput")
    wfc_d = nc.dram_tensor("wfc", [HID // P, P, DC, P], F16,
                           kind="ExternalInput")
    bfc_d = nc.dram_tensor("bfc", [P, HID // P], F32, kind="ExternalInput")
    wfc2_d = nc.dram_tensor("wfc2", [HID, D], F16, kind="ExternalInput")
    bfc2b_d = nc.dram_tensor("bfc2b", [P, D], F32, kind="ExternalInput")
    out_d = nc.dram_tensor("out", [TS, D], F32, kind="ExternalOutput")

    rs_in = [nc.dram_tensor(f"rs_in{q}", [SPAN, D], F16) for q in range(NQ)]
    rs_out = [nc.dram_tensor(f"rs_out{q}", [P, D], F16) for q in range(NQ)]

    with tile.TileContext(nc) as tc:
        cst_cm = tc.tile_pool(name="cst", bufs=1)
        cst = cst_cm.__enter__()
        big_cm = tc.tile_pool(name="big", bufs=1)
        big = big_cm.__enter__()

        # ---- first xT tiles before anything else hits the DMA queues ----
        pxt_cm = tc.tile_pool(name="p_xt", bufs=2)
        p_xt = pxt_cm.__enter__()
        xts = []
        xt0 = p_xt.tile([P, DC, SPAN], F16, tag="xt", name="xt0")
        nc.sync.dma_start(xt0[:], xt_d[:, :, 0:SPAN].rearrange("c p t -> p c t"))
        xts.append(xt0)

        # ---- constants / small params ----
        identf = cst.tile([P, P], F32)
        make_identity(nc, identf[:])
        ident = cst.tile([P, P], F16)
        nc.vector.tensor_copy(ident[:], identf[:])
        ones4 = cst.tile([P, FH, 1], F16)
        with nc.allow_low_precision("exact value 1.0"):
            nc.gpsimd.memset(ones4[:], 1.0)
        ones1 = cst.tile([P, 1], F16)
        with nc.allow_low_precision("exact value 1.0"):
            nc.gpsimd.memset(ones1[:], 1.0)
        epsb = cst.tile([P, 1], F32)
        nc.gpsimd.memset(epsb[:], EPS)

        bqkt = cst.tile([P, 4], F32)
        nc.sync.dma_start(bqkt[:], bqk_d[:])
        bfct = cst.tile([P, HID // P], F32)
        nc.sync.dma_start(bfct[:], bfc_d[:])
        bvb = cst.tile([P, FQ], F32)
        nc.sync.dma_start(bvb[:], bvb_d[:])
        bfc2b = cst.tile([P, D], F32)
        nc.sync.dma_start(bfc2b[:], bfc2b_d[:])

        # ---- resident weights ----
        wq_t = big.tile([P, DC, FQ], F16, name="wq_t")
        wk_t = big.tile([P, DC, FQ], F16, name="wk_t")
        wv_t = big.tile([P, DC, FQ], F16, name="wv_t")
        wp_t = big.tile([P, 2, D], F16, name="wp_t")
        nc.sync.dma_start(wq_t[:], wq_d[:, :].rearrange("(c p) f -> p c f", p=P))
        nc.sync.dma_start(wk_t[:], wk_d[:, :].rearrange("(c p) f -> p c f", p=P))
        nc.sync.dma_start(wv_t[:], wv_d[:, :].rearrange("(c p) f -> p c f", p=P))
        nc.sync.dma_start(wp_t[:], wp_d[:, :].rearrange("(c p) f -> p c f", p=P))
        # fc1 weight, fully preloaded (DMA issued at attention start);
        # lives in `big` so it survives until the FFN phase
        wfc_sb = big.tile([P, HID // P, DC, P], F16, name="wfc_sb")

        # ---- big tag-shared tiles (front/attention lives) ----
        h1T = big.tile([P, DC, T], F16, tag="A", name="h1T")
        qT = big.tile([P, 2, T], F16, tag="B1", name="qT")
        kT = big.tile([P, 2, T], F16, tag="B2", name="kT")
        vhat = big.tile([P, NT, FH * (DH + 1)], F16, tag="B3", name="vhat")
        attT = big.tile([P, 2, T], F16, tag="B4", name="attT")

        # ============ Front: LN1 (d-major) + QKV, software-pipelined =======
        qk_meta = [(wq_t, 0, qT, 0), (wq_t, 1, qT, 1),
                   (wk_t, 0, kT, 0), (wk_t, 1, kT, 1)]
        with tc.tile_pool(name="p_sq", bufs=3) as p_sq, \
             tc.tile_pool(name="p_ln", bufs=2) as p_ln, \
             tc.tile_pool(name="ps_st", bufs=2, space="PSUM") as ps_st, \
             tc.tile_pool(name="ps_qk", bufs=2, space="PSUM") as ps_qk, \
             tc.tile_pool(name="ps_v", bufs=2, space="PSUM") as ps_v:

            def stats(s):
                """Token stats for span s -> normalized xhat_T into h1T."""
                if s == 0:
                    xt = xts[0]
                else:
                    xt = p_xt.tile([P, DC, SPAN], F16, tag="xt",
                                   name=f"xt{s}")
                    nc.sync.dma_start(
                        xt[:],
                        xt_d[:, :, s * SPAN:(s + 1) * SPAN]
                        .rearrange("c p t -> p c t"))
                    xts.append(xt)
                sums = ps_st.tile([1, SPAN], F32, tag="sums", name=f"sums{s}")
                sqs = ps_st.tile([33, SPAN], F32, tag="sqs", name=f"sqs{s}")
                for c in range(DC):
                    xsq = p_sq.tile([P, SPAN], F16, tag="xsq",
                                    name=f"xsq{s}_{c}")
                    with nc.allow_low_precision("fp16 activations"):
                        nc.scalar.activation(xsq[:], xt[:, c, :], AF.Square)
                    nc.tensor.matmul(sums[0:1, :], ones1[:], xt[:, c, :],
                                     start=(c == 0), stop=(c == DC - 1))
                    nc.tensor.matmul(sqs[32:33, :], ones1[:], xsq[:],
                                     start=(c == 0), stop=(c == DC - 1))
                # mu = sum/D ; var = sumsq/D - mu^2 ; rstd = (var+eps)^-1/2
                st = p_ln.tile([1, 5, SPAN], F32, tag="st", bufs=1,
                               name=f"st{s}")
                mu, var, sd, rstd, mrs = (st[:, i, :] for i in range(5))
                nc.vector.tensor_scalar(mu, sums[0:1, :], 1.0 / D, None,
                                        OP.mult)
                nc.vector.scalar_tensor_tensor(var, mu, -1.0, mu,
                                               op0=OP.mult, op1=OP.mult)
                nc.vector.scalar_tensor_tensor(var, sqs[32:33, :], 1.0 / D,
                                               var, op0=OP.mult, op1=OP.add)
                nc.scalar.activation(sd, var, AF.Sqrt, bias=epsb[0:1, :])
                nc.vector.reciprocal(rstd, sd)
                nc.vector.tensor_tensor(mrs, mu, rstd, OP.mult)
                rows = p_ln.tile([1, 2, SPAN], F16, tag="rows", bufs=1,
                                 name=f"rows{s}")
                with nc.allow_low_precision("fp16 activations"):
                    nc.vector.tensor_copy(rows[:, 0, :], rstd)
                    nc.vector.tensor_copy(rows[:, 1, :], mrs)
                bc = p_ln.tile([P, 2, SPAN], F16, tag="bc", name=f"bc{s}")
                nc.gpsimd.partition_broadcast(bc[:, 0, :], rows[:, 0, :])
                nc.gpsimd.partition_broadcast(bc[:, 1, :], rows[:, 1, :])
                # xhat_T = xT * R - M
                with nc.allow_low_precision("fp16 activations"):
                    for c in range(DC):
                        nc.vector.tensor_tensor(
                            h1T[:, c, s * SPAN:(s + 1) * SPAN],
                            xt[:, c, :], bc[:, 0, :], OP.mult)
                        nc.vector.tensor_tensor(
                            h1T[:, c, s * SPAN:(s + 1) * SPAN],
                            h1T[:, c, s * SPAN:(s + 1) * SPAN],
                            bc[:, 1, :], OP.subtract)

            def qkv(s):
                for fb in range(4):
                    wsrc, half, dest, dhalf = qk_meta[fb]
                    pq = ps_qk.tile([P, SPAN], F32, tag="qk",
                                    name=f"qk{fb}_{s}")
                    for kc in range(DC):
                        nc.tensor.matmul(
                            pq[:], wsrc[:, kc, half * P:(half + 1) * P],
                            h1T[:, kc, s * SPAN:(s + 1) * SPAN],
                            start=(kc == 0), stop=(kc == DC - 1))
                    with nc.allow_low_precision("fp16 activations"):
                        nc.scalar.activation(
                            dest[:, dhalf, s * SPAN:(s + 1) * SPAN], pq[:],
                            AF.Identity, bias=bqkt[:, fb:fb + 1])
                for m in range(4 * s, 4 * s + 4):
                    pvt = ps_v.tile([P, 512], F32, tag="v", name=f"v{m}")
                    pv = pvt[:, 0:FQ]
                    for kc in range(DC):
                        nc.tensor.matmul(
                            pv, h1T[:, kc, m * P:(m + 1) * P],
                            wv_t[:, kc, :],
                            start=(kc == 0), stop=(kc == DC - 1))
                    vdst = vhat[:, m, :].rearrange("p (h x) -> p h x",
                                                   x=DH + 1)
                    with nc.allow_low_precision("fp16 activations"):
                        nc.vector.tensor_tensor(
                            vdst[:, :, 0:DH],
                            pv.rearrange("p (h x) -> p h x", x=DH),
                            bvb[:].rearrange("p (h x) -> p h x", x=DH),
                            OP.add)
                        nc.vector.tensor_copy(vdst[:, :, DH:DH + 1], ones4[:])

            stats(0)
            for s in range(NSPAN):
                if s + 1 < NSPAN:
                    stats(s + 1)
                qkv(s)
        pxt_cm.__exit__(None, None, None)

        # prefetch fc1 weights into SBUF while attention computes
        for pq4 in range(4):
            m0 = pq4 * (HID // P // 4)
            m1 = (pq4 + 1) * (HID // P // 4)
            nc.sync.dma_start(
                wfc_sb[:, m0:m1, :, :],
                wfc_d[m0:m1].rearrange("m p c f -> p m c f"))

        # ---- phase-5 persistent tiles ----
        x2 = big.tile([P, NQ, D], F16, tag="B1x", name="x2")
        h2T = big.tile([P, DC, TS], F16, tag="B2x", name="h2T")
        mvq = big.tile([P, NQ, 2], F32, name="mvq")

        # per-quarter residual + LN2 stats, DVE-only (runs during attention)
        def p5_early(q, p_rs, p_l2):
            rst = p_rs.tile([P, D], F16, tag="rst", name=f"rst{q}")
            nc.sync.dma_start(rst[:], rs_out[q][:])
            xsq_ = p_rs.tile([P, D], F16, tag="xsl", name=f"xsl{q}")
            nc.sync.dma_start(xsq_[:], xs_d[q * P:(q + 1) * P, :])
            with nc.allow_low_precision("fp16 residual"):
                nc.vector.tensor_tensor(x2[:, q, :], rst[:], xsq_[:], OP.add)
            bn6 = p_l2.tile([P, 2, 6], F32, tag="bn6", name=f"bn6_{q}")
            for a in range(2):
                nc.vector.bn_stats(bn6[:, a, :],
                                   x2[:, q, a * 512:(a + 1) * 512])
            nc.vector.bn_aggr(mvq[:, q, :], bn6[:])

        # normalize + transpose (post-attention; Sqrt shares no exp table)
        def p5_late(q, p_l2, ps_t2):
            sd = p_l2.tile([P, 1], F32, tag="sd2", name=f"sd2_{q}")
            nc.scalar.activation(sd[:], mvq[:, q, 1:2], AF.Sqrt,
                                 bias=epsb[:])
            rstd = p_l2.tile([P, 1], F32, tag="rstd2", name=f"rstd2_{q}")
            nc.vector.reciprocal(rstd[:], sd[:])
            xh2 = p_l2.tile([P, D], F16, tag="xh2", name=f"xh2_{q}")
            with nc.allow_low_precision("fp16 activations"):
                nc.vector.tensor_scalar(
                    xh2[:], x2[:, q, :], mvq[:, q, 0:1], rstd[:],
                    OP.subtract, OP.mult)
            for j in range(DC):
                ptt = ps_t2.tile([P, 8, P], F16, tag="t2", name=f"t2_{q}_{j}")
                pt = ptt[:, 0, :]
                nc.tensor.transpose(pt, xh2[:, j * P:(j + 1) * P],
                                    ident[:])
                with nc.allow_low_precision("fp16 activations"):
                    nc.vector.tensor_copy(h2T[:, j, q * P:(q + 1) * P], pt)

        # ================= Attention + proj + ReduceScatter =================
        with tc.tile_pool(name="p_e", bufs=1) as p_e, \
             tc.tile_pool(name="p_pr", bufs=3) as p_pr, \
             tc.tile_pool(name="ps_s", bufs=2, space="PSUM") as ps_s, \
             tc.tile_pool(name="ps_pv", bufs=2, space="PSUM") as ps_pv:
            for s in range(NSPAN):
                nkb = (s + 1) * (SPAN // P)
                for hp in range(2):
                    ppvs = [ps_pv.tile([DH + 1, SPAN], F32, tag=f"pv{i}",
                                       name=f"pv{hp * 2 + i}_{s}")
                            for i in range(2)]
                    es = {}

                    def scores(kb, s=s, hp=hp, es=es):
                        j = kb - s * (SPAN // P)
                        c0 = max(0, j) * P   # first valid query col
                        pst = ps_s.tile([P, 2, SPAN], F32, tag="sT",
                                        name=f"sT{hp}_{s}_{kb}")
                        for hh in range(2):
                            nc.tensor.matmul(
                                pst[:, hh, c0:SPAN],
                                kT[hh * DH:(hh + 1) * DH, hp,
                                   kb * P:(kb + 1) * P],
                                qT[hh * DH:(hh + 1) * DH, hp,
                                   s * SPAN + c0:(s + 1) * SPAN])
                        e = p_e.tile([P, 2, SPAN], F16, tag="e",
                                     bufs=SKEW + 2, name=f"e{hp}_{s}_{kb}")
                        with nc.allow_low_precision("fp16 activations"):
                            nc.scalar.activation(e[:, :, c0:SPAN],
                                                 pst[:, :, c0:SPAN],
                                                 AF.Exp, scale=0.125)
                        if j >= 0:
                            # zero the causal triangle of the diagonal block
                            nc.gpsimd.affine_select(
                                out=e[:, :, c0:c0 + P],
                                in_=e[:, :, c0:c0 + P],
                                compare_op=OP.is_ge, fill=0.0, base=0,
                                channel_multiplier=-1,
                                pattern=[[0, 2], [1, P]])
                        es[kb] = (e, c0)

                    def pv_acc(kb, s=s, hp=hp, ppvs=ppvs, nkb=nkb, es=es):
                        e, c0 = es.pop(kb)
                        for hh in range(2):
                            h = hp * 2 + hh
                            nc.tensor.matmul(
                                ppvs[hh][:, c0:SPAN],
                                vhat[:, kb, h * (DH + 1):(h + 1) * (DH + 1)],
                                e[:, hh, c0:SPAN], start=(kb == 0),
                                stop=(kb == nkb - 1))

                    for i in range(nkb + SKEW):
                        if i < nkb:
                            scores(i)
                        if i >= SKEW:
                            pv_acc(i - SKEW)
                    # normalize: att = pv / den (recip of den row, bcast)
                    for hh in range(2):
                        ppv = ppvs[hh]
                        den = p_e.tile([1, SPAN], F32, tag="den", bufs=2,
                                       name=f"den{hp}_{hh}_{s}")
                        nc.vector.reciprocal(den[:], ppv[DH:DH + 1, :])
                        rbs = p_e.tile([DH, SPAN], F32, tag="rbs", bufs=2,
                                       name=f"rbs{hp}_{hh}_{s}")
                        nc.gpsimd.partition_broadcast(rbs[:], den[:],
                                                      channels=DH)
                        with nc.allow_low_precision("fp16 activations"):
                            nc.vector.tensor_tensor(
                                attT[hh * DH:(hh + 1) * DH, hp,
                                     s * SPAN:(s + 1) * SPAN],
                                ppv[0:DH, :], rbs[:], OP.mult)

                # projection partial for this token quarter, then its RS
                for mtl in range(4):
                    m = s * 4 + mtl
                    pp = ps_s.tile([P, 2, SPAN], F32, tag="sT",
                                   name=f"pr{m}")
                    for kc in range(2):
                        for n in range(2):
                            nc.tensor.matmul(
                                pp[:, n, :], attT[:, kc, m * P:(m + 1) * P],
                                wp_t[:, kc, n * SPAN:(n + 1) * SPAN],
                                start=(kc == 0), stop=(kc == 1))
                    pe = p_pr.tile([P, D], F16, tag="pe", name=f"pe{m}")
                    with nc.allow_low_precision("fp16 rs payload"):
                        nc.vector.tensor_copy(pe[:], pp[:])
                    nc.sync.dma_start(
                        rs_in[s][mtl * P:(mtl + 1) * P, :], pe[:])
                nc.gpsimd.collective_compute(
                    "ReduceScatter", OP.add, replica_groups=GROUPS,
                    ins=[rs_in[s][:]], outs=[rs_out[s][:]])

        # ================= FFN =================
        # aT reuses the dead qT/kT/vhat/attT slots (4 x 8 hid-tiles)
        aTs = [big.tile([P, HID // P // 4, TS], F16, tag=f"B{i + 1}",
                        name=f"aT{i}") for i in range(4)]

        def aT(m):
            return aTs[m // 8][:, m % 8, :]

        with tc.tile_pool(name="p_rs", bufs=2) as p_rs, \
             tc.tile_pool(name="p_l2", bufs=2) as p_l2, \
             tc.tile_pool(name="ps_t2", bufs=2, space="PSUM") as ps_t2, \
             tc.tile_pool(name="ps_f1", bufs=4, space="PSUM") as ps_f1:

            def fc1_half(ha):
                t0, t1 = ha * 256, (ha + 1) * 256
                for m in range(HID // P):
                    pft = ps_f1.tile([P, 512], F32, tag="f1",
                                     name=f"f1_{ha}_{m}")
                    pf = pft[:, 0:256]
                    for kc in range(DC):
                        nc.tensor.matmul(pf, wfc_sb[:, m, kc, :],
                                         h2T[:, kc, t0:t1],
                                         start=(kc == 0), stop=(kc == DC - 1))
                    with nc.allow_low_precision("fp16 activations"):
                        nc.scalar.activation(aT(m)[:, t0:t1], pf, AF.Gelu,
                                             bias=bfct[:, m:m + 1])

            for q in range(2):
                p5_early(q, p_rs, p_l2)
                p5_late(q, p_l2, ps_t2)
            fc1_half(0)
            for q in range(2, NQ):
                p5_early(q, p_rs, p_l2)
                p5_late(q, p_l2, ps_t2)
            fc1_half(1)

        with tc.tile_pool(name="p_w2", bufs=6) as p_w2, \
             tc.tile_pool(name="p_ot", bufs=3) as p_ot, \
             tc.tile_pool(name="ps_f2", bufs=4, space="PSUM") as ps_f2:
            p4s = [ps_f2.tile([P, 2, SPAN], F32, tag="f2", name=f"f2_{mt}")
                   for mt in range(NQ)]
            for kc in range(HID // P):
                w2 = p_w2.tile([P, D], F16, tag="w2", name=f"w2_{kc}")
                nc.sync.dma_start(w2[:], wfc2_d[kc * P:(kc + 1) * P, :])
                for mt in range(NQ):
                    for n in range(2):
                        nc.tensor.matmul(
                            p4s[mt][:, n, :],
                            aT(kc)[:, mt * P:(mt + 1) * P],
                            w2[:, n * SPAN:(n + 1) * SPAN],
                            start=(kc == 0), stop=(kc == HID // P - 1))
            for mt in range(NQ):
                for n in range(2):
                    ot = p_ot.tile([P, SPAN], F32, tag="ot",
                                   name=f"ot{n}_{mt}")
                    nc.vector.tensor_tensor(
                        ot[:], p4s[mt][:, n, :],
                        x2[:, mt, n * SPAN:(n + 1) * SPAN], OP.add)
                    nc.vector.tensor_tensor(
                        ot[:], ot[:],
                        bfc2b[:, n * SPAN:(n + 1) * SPAN], OP.add)
                    nc.sync.dma_start(
                        out_d[mt * P:(mt + 1) * P, n * SPAN:(n + 1) * SPAN],
                        ot[:])

        big_cm.__exit__(None, None, None)
        cst_cm.__exit__(None, None, None)

    nc.finalize()
    return nc


def shard_inputs(inputs):
    """Full inputs -> per-core in_maps (8 cores)."""
    f = lambda a: np.ascontiguousarray(np.asarray(a, dtype=np.float32))
    h = lambda a: np.ascontiguousarray(np.asarray(a, dtype=np.float16))
    x = f(inputs["x"])
    w_attn, b_attn = f(inputs["w_attn"]), f(inputs["b_attn"])
    w_proj, b_proj = f(inputs["w_proj"]), f(inputs["b_proj"])
    ln1_g, ln1_b = f(inputs["ln1_g"]), f(inputs["ln1_b"])
    ln2_g, ln2_b = f(inputs["ln2_g"]), f(inputs["ln2_b"])
    w_fc, b_fc = f(inputs["w_fc"]), f(inputs["b_fc"])
    w_fc2, b_fc2 = f(inputs["w_fc2"]), f(inputs["b_fc2"])

    # fold LN1 gamma into W_attn rows, beta into the qkv bias
    # LN(x)@W = xhat@(g*W) + b@W  (b uses the unscaled W)
    w_attn_f = w_attn * ln1_g[:, None]
    b_attn_f = b_attn + ln1_b @ w_attn
    # fold LN2 gamma into W_fc rows, beta into b_fc
    w_fc_f = w_fc * ln2_g[:, None]
    b_fc_f = b_fc + ln2_b @ w_fc

    wfc_r = h(w_fc_f.reshape(DC, P, HID // P, P).transpose(2, 1, 0, 3))
    bfct = np.ascontiguousarray(b_fc_f.reshape(HID // P, P).T)
    bfc2b = np.ascontiguousarray(np.broadcast_to(b_fc2.reshape(1, D), (P, D)))
    wfc2_h = h(w_fc2)

    in_maps = []
    for c in range(8):
        g, tp = c // 4, c % 4
        sl = slice(tp * FQ, (tp + 1) * FQ)
        bq = b_attn_f[0 * D:1 * D][sl]
        bk = b_attn_f[1 * D:2 * D][sl]
        bv = b_attn_f[2 * D:3 * D][sl]
        bqk = np.concatenate([bq.reshape(2, P).T, bk.reshape(2, P).T], axis=1)
        bvb = np.broadcast_to(bv.reshape(1, FQ), (P, FQ))
        xs = np.concatenate(
            [x[g, q * SPAN + tp * P: q * SPAN + (tp + 1) * P]
             for q in range(NQ)], axis=0) + b_proj.reshape(1, D)
        xt = np.ascontiguousarray(x[g].T.reshape(DC, P, T))
        in_maps.append({
            "xt": h(xt),
            "xs": h(xs),
            "wq": h(w_attn_f[:, 0 * D:1 * D][:, sl]),
            "wk": h(w_attn_f[:, 1 * D:2 * D][:, sl]),
            "wv": h(w_attn_f[:, 2 * D:3 * D][:, sl]),
            "bqk": np.ascontiguousarray(bqk),
            "bvb": np.ascontiguousarray(bvb),
            "wp": h(w_proj[sl, :]),
            "wfc": wfc_r, "bfc": bfct,
            "wfc2": wfc2_h, "bfc2b": bfc2b,
        })
    return in_maps


def assemble(results):
    out = np.empty((2, T, D), dtype=np.float32)
    for c in range(8):
        g, tp = c // 4, c % 4
        r = np.asarray(results[c]["out"])
        for q in range(NQ):
            out[g, q * SPAN + tp * P: q * SPAN + (tp + 1) * P] = \
                r[q * P:(q + 1) * P]
    return out


_NC = None


def kernel(**inputs):
    global _NC
    if _NC is None:
        _NC = build_nc()
    in_maps = shard_inputs(inputs)
    res = run_bass_kernel_spmd(_NC, in_maps, list(range(8)))
    return assemble(res.results)


# revision 19
# speedup vs baseline: 1.0852x; 1.0000x over previous
"""Trainium2 Bass kernel for a GPT-style decoder block (B=2, T=2048, d=1024,
16 heads, FFN 4096), distributed over 8 NeuronCores.

Sharding: DP2 (batch) x TP4 (4 heads + proj-row split per core). The single
collective is a per-token-quarter ReduceScatter of the attention projection
partials over each 4-core group; after it, every core owns its token strips
and runs LN2+FFN (full hidden dim) on just those, writing its 512-token
output slice.

v4 vs v3 baseline:
- x staged pre-transposed (xT) from host; LN1 computed in d-major layout via
  ones-matmul token stats + broadcast rows (eliminates 128 PE transposes).
- LN1 gamma/beta folded into W_qkv / qkv biases; LN2 gamma/beta folded into
  W_fc / b_fc (host-side constant prep).
- Scores per head-pair packed into one [128,2,512] PSUM tile via row-tiled
  concurrent matmuls; one exp per pair-block; col-restricted exp/scores/PV
  on causal-diagonal blocks.
- qkv bias adds on DVE; rstd via gpsimd pow -> ACT does only Exp + Gelu
  (no activation-table thrash).
- Per-quarter residual+LN2 inline right after each ReduceScatter lands
  (overlaps attention).
- Front phase software-pipelined: span s+1 token stats run on PE before
  span s QKV so the LN broadcast chain hides under matmuls.

Self-contained: hardcodes all shapes; no sibling imports.
"""
import numpy as np

import concourse.bacc as bacc
import concourse.mybir as mybir
import concourse.tile as tile
from concourse.bass_utils import run_bass_kernel_spmd
from concourse.masks import make_identity

F32 = mybir.dt.float32
F16 = mybir.dt.float16
AF = mybir.ActivationFunctionType
OP = mybir.AluOpType

P = 128
T = 2048          # tokens per batch element
D = 1024          # embed dim
NT = T // P       # 16 token tiles
DC = D // P       # 8 d-chunks
FH = 4            # heads per core
DH = 64           # head dim
FQ = 256          # q (=k=v) features per core
HID = 4096        # full FFN hidden
TS = 512          # token slice per core
NQ = 4            # token quarters
SPAN = 512        # attention query span
NSPAN = T // SPAN
EPS = 1e-5
GROUPS = [[0, 1, 2, 3], [4, 5, 6, 7]]
SKEW = 3          # score pair-blocks run this far ahead of the PV chain


def build_nc():
    nc = bacc.Bacc(None, target_bir_lowering=False)

    # ---- external I/O ----
    xt_d = nc.dram_tensor("xt", [DC, P, T], F16, kind="ExternalInput")
    xs_d = nc.dram_tensor("xs", [TS, D], F16, kind="ExternalInput")
    wq_d = nc.dram_tensor("wq", [D, FQ], F16, kind="ExternalInput")
    wk_d = nc.dram_tensor("wk", [D, FQ], F16, kind="ExternalInput")
    wv_d = nc.dram_tensor("wv", [D, FQ], F16, kind="ExternalInput")
    bqk_d = nc.dram_tensor("bqk", [P, 4], F32, kind="ExternalInput")
    bvb_d = nc.dram_tensor("bvb", [P, FQ], F32, kind="ExternalInput")
    wp_d = nc.dram_tensor("wp", [FQ, D], F16, kind="ExternalInput")
    wfc_d = nc.dram_tensor("wfc", [HID // P, P, DC, P], F16,
                           kind="ExternalInput")
    bfc_d = nc.dram_tensor("bfc", [P, HID // P], F32, kind="ExternalInput")
    wfc2_d = nc.dram_tensor("wfc2", [HID, D], F16, kind="ExternalInput")
    bfc2b_d = nc.dram_tensor("bfc2b", [P, D], F32, kind="ExternalInput")
    out_d = nc.dram_tensor("out", [TS, D], F32, kind="ExternalOutput")

    rs_in = [nc.dram_tensor(f"rs_in{q}", [SPAN, D], F16) for q in range(NQ)]
    rs_out = [nc.dram_tensor(f"rs_out{q}", [P, D], F16) for q in range(NQ)]

    with tile.TileContext(nc) as tc:
        cst_cm = tc.tile_pool(name="cst", bufs=1)
        cst = cst_cm.__enter__()
        big_cm = tc.tile_pool(name="big", bufs=1)
        big = big_cm.__enter__()

        # ---- first xT tiles before anything else hits the DMA queues ----
        pxt_cm = tc.tile_pool(name="p_xt", bufs=2)
        p_xt = pxt_cm.__enter__()
        xts = []
        xt0 = p_xt.tile([P, DC, SPAN], F16, tag="xt", name="xt0")
        nc.sync.dma_start(xt0[:], xt_d[:, :, 0:SPAN].rearrange("c p t -> p c t"))
        xts.append(xt0)

        # ---- constants / small params ----
        identf = cst.tile([P, P], F32)
        make_identity(nc, identf[:])
        ident = cst.tile([P, P], F16)
        nc.vector.tensor_copy(ident[:], identf[:])
        ones4 = cst.tile([P, FH, 1], F16)
        with nc.allow_low_precision("exact value 1.0"):
            nc.gpsimd.memset(ones4[:], 1.0)
        ones1 = cst.tile([P, 1], F16)
        with nc.allow_low_precision("exact value 1.0"):
            nc.gpsimd.memset(ones1[:], 1.0)
        epsb = cst.tile([P, 1], F32)
        nc.gpsimd.memset(epsb[:], EPS)

        bqkt = cst.tile([P, 4], F32)
        nc.sync.dma_start(bqkt[:], bqk_d[:])
        bfct = cst.tile([P, HID // P], F32)
        nc.sync.dma_start(bfct[:], bfc_d[:])
        bvb = cst.tile([P, FQ], F32)
        nc.sync.dma_start(bvb[:], bvb_d[:])
        bfc2b = cst.tile([P, D], F32)
        nc.sync.dma_start(bfc2b[:], bfc2b_d[:])

        # ---- resident weights ----
        wq_t = big.tile([P, DC, FQ], F16, name="wq_t")
        wk_t = big.tile([P, DC, FQ], F16, name="wk_t")
        wv_t = big.tile([P, DC, FQ], F16, name="wv_t")
        wp_t = big.tile([P, 2, D], F16, name="wp_t")
        nc.sync.dma_start(wq_t[:], wq_d[:, :].rearrange("(c p) f -> p c f", p=P))
        nc.sync.dma_start(wk_t[:], wk_d[:, :].rearrange("(c p) f -> p c f", p=P))
        nc.sync.dma_start(wv_t[:], wv_d[:, :].rearrange("(c p) f -> p c f", p=P))
        nc.sync.dma_start(wp_t[:], wp_d[:, :].rearrange("(c p) f -> p c f", p=P))
        # fc1 weight, fully preloaded (DMA issued at attention start);
        # lives in `big` so it survives until the FFN phase
        wfc_sb = big.tile([P, HID // P, DC, P], F16, name="wfc_sb")

        # ---- big tag-shared tiles (front/attention lives) ----
        h1T = big.tile([P, DC, T], F16, tag="A", name="h1T")
        qT = big.tile([P, 2, T], F16, tag="B1", name="qT")
        kT = big.tile([P, 2, T], F16, tag="B2", name="kT")
        vhat = big.tile([P, NT, FH * (DH + 1)], F16, tag="B3", name="vhat")
        attT = big.tile([P, 2, T], F16, tag="B4", name="attT")

        # ============ Front: LN1 (d-major) + QKV, software-pipelined =======
        qk_meta = [(wq_t, 0, qT, 0), (wq_t, 1, qT, 1),
                   (wk_t, 0, kT, 0), (wk_t, 1, kT, 1)]
        with tc.tile_pool(name="p_sq", bufs=3) as p_sq, \
             tc.tile_pool(name="p_ln", bufs=2) as p_ln, \
             tc.tile_pool(name="ps_st", bufs=2, space="PSUM") as ps_st, \
             tc.tile_pool(name="ps_qk", bufs=2, space="PSUM") as ps_qk, \
             tc.tile_pool(name="ps_v", bufs=2, space="PSUM") as ps_v:

            def stats(s):
                """Token stats for span s -> normalized xhat_T into h1T."""
                if s == 0:
                    xt = xts[0]
                else:
                    xt = p_xt.tile([P, DC, SPAN], F16, tag="xt",
                                   name=f"xt{s}")
                    nc.sync.dma_start(
                        xt[:],
                        xt_d[:, :, s * SPAN:(s + 1) * SPAN]
                        .rearrange("c p t -> p c t"))
                    xts.append(xt)
                sums = ps_st.tile([1, SPAN], F32, tag="sums", name=f"sums{s}")
                sqs = ps_st.tile([33, SPAN], F32, tag="sqs", name=f"sqs{s}")
                for c in range(DC):
                    xsq = p_sq.tile([P, SPAN], F16, tag="xsq",
                                    name=f"xsq{s}_{c}")
                    with nc.allow_low_precision("fp16 activations"):
                        nc.scalar.activation(xsq[:], xt[:, c, :], AF.Square)
                    nc.tensor.matmul(sums[0:1, :], ones1[:], xt[:, c, :],
                                     start=(c == 0), stop=(c == DC - 1))
                    nc.tensor.matmul(sqs[32:33, :], ones1[:], xsq[:],
                                     start=(c == 0), stop=(c == DC - 1))
                # mu = sum/D ; var = sumsq/D - mu^2 ; rstd = (var+eps)^-1/2
                st = p_ln.tile([1, 5, SPAN], F32, tag="st", bufs=1,
                               name=f"st{s}")
                mu, var, sd, rstd, mrs = (st[:, i, :] for i in range(5))
                nc.vector.tensor_scalar(mu, sums[0:1, :], 1.0 / D, None,
                                        OP.mult)
                nc.vector.scalar_tensor_tensor(var, mu, -1.0, mu,
                                               op0=OP.mult, op1=OP.mult)
                nc.vector.scalar_tensor_tensor(var, sqs[32:33, :], 1.0 / D,
                                               var, op0=OP.mult, op1=OP.add)
                nc.scalar.activation(sd, var, AF.Sqrt, bias=epsb[0:1, :])
                mu16 = p_ln.tile([1, SPAN], F16, tag="mu16", bufs=1,
                                 name=f"mu16_{s}")
                with nc.allow_low_precision("fp16 activations"):
                    nc.vector.tensor_copy(mu16[:], mu)
                bcu = p_ln.tile([P, SPAN], F16, tag="bcu", name=f"bcu{s}")
                nc.gpsimd.partition_broadcast(bcu[:], mu16[:])
                bcs = p_ln.tile([P, SPAN], F32, tag="bcs", name=f"bcs{s}")
                nc.gpsimd.partition_broadcast(bcs[:], sd)
                rs16 = p_ln.tile([P, SPAN], F16, tag="rs16", name=f"rs16_{s}")
                with nc.allow_low_precision("fp16 activations"):
                    nc.vector.reciprocal(rs16[:], bcs[:])
                # xhat_T = (xT - U) * (1/S)
                with nc.allow_low_precision("fp16 activations"):
                    for c in range(DC):
                        nc.vector.tensor_tensor(
                            h1T[:, c, s * SPAN:(s + 1) * SPAN],
                            xt[:, c, :], bcu[:], OP.subtract)
                        nc.vector.tensor_tensor(
                            h1T[:, c, s * SPAN:(s + 1) * SPAN],
                            h1T[:, c, s * SPAN:(s + 1) * SPAN],
                            rs16[:], OP.mult)

            def qkv(s):
                for fb in range(4):
                    wsrc, half, dest, dhalf = qk_meta[fb]
                    pq = ps_qk.tile([P, SPAN], F32, tag="qk",
                                    name=f"qk{fb}_{s}")
                    for kc in range(DC):
                        nc.tensor.matmul(
                            pq[:], wsrc[:, kc, half * P:(half + 1) * P],
                            h1T[:, kc, s * SPAN:(s + 1) * SPAN],
                            start=(kc == 0), stop=(kc == DC - 1))
                    with nc.allow_low_precision("fp16 activations"):
                        nc.scalar.activation(
                            dest[:, dhalf, s * SPAN:(s + 1) * SPAN], pq[:],
                            AF.Identity, bias=bqkt[:, fb:fb + 1])
                for m in range(4 * s, 4 * s + 4):
                    pvt = ps_v.tile([P, 512], F32, tag="v", name=f"v{m}")
                    pv = pvt[:, 0:FQ]
                    for kc in range(DC):
                        nc.tensor.matmul(
                            pv, h1T[:, kc, m * P:(m + 1) * P],
                            wv_t[:, kc, :],
                            start=(kc == 0), stop=(kc == DC - 1))
                    vdst = vhat[:, m, :].rearrange("p (h x) -> p h x",
                                                   x=DH + 1)
                    with nc.allow_low_precision("fp16 activations"):
                        nc.vector.tensor_tensor(
                            vdst[:, :, 0:DH],
                            pv.rearrange("p (h x) -> p h x", x=DH),
                            bvb[:].rearrange("p (h x) -> p h x", x=DH),
                            OP.add)
                        nc.vector.tensor_copy(vdst[:, :, DH:DH + 1], ones4[:])

            stats(0)
            for s in range(NSPAN):
                if s + 1 < NSPAN:
                    stats(s + 1)
                qkv(s)
        pxt_cm.__exit__(None, None, None)

        # prefetch fc1 weights into SBUF while attention computes
        for pq4 in range(4):
            m0 = pq4 * (HID // P // 4)
            m1 = (pq4 + 1) * (HID // P // 4)
            nc.sync.dma_start(
                wfc_sb[:, m0:m1, :, :],
                wfc_d[m0:m1].rearrange("m p c f -> p m c f"))

        # ---- phase-5 persistent tiles ----
        x2 = big.tile([P, NQ, D], F16, tag="B1x", name="x2")
        h2T = big.tile([P, DC, TS], F16, tag="B2x", name="h2T")
        mvq = big.tile([P, NQ, 2], F32, name="mvq")

        # per-quarter residual + LN2 stats, DVE-only (runs during attention)
        def p5_early(q, p_rs, p_l2):
            rst = p_rs.tile([P, D], F16, tag="rst", name=f"rst{q}")
            nc.sync.dma_start(rst[:], rs_out[q][:])
            xsq_ = p_rs.tile([P, D], F16, tag="xsl", name=f"xsl{q}")
            nc.sync.dma_start(xsq_[:], xs_d[q * P:(q + 1) * P, :])
            with nc.allow_low_precision("fp16 residual"):
                nc.vector.tensor_tensor(x2[:, q, :], rst[:], xsq_[:], OP.add)
            bn6 = p_l2.tile([P, 2, 6], F32, tag="bn6", name=f"bn6_{q}")
            for a in range(2):
                nc.vector.bn_stats(bn6[:, a, :],
                                   x2[:, q, a * 512:(a + 1) * 512])
            nc.vector.bn_aggr(mvq[:, q, :], bn6[:])

        # normalize + transpose (post-attention; Sqrt shares no exp table)
        def p5_late(q, p_l2, ps_t2):
            sd = p_l2.tile([P, 1], F32, tag="sd2", name=f"sd2_{q}")
            nc.scalar.activation(sd[:], mvq[:, q, 1:2], AF.Sqrt,
                                 bias=epsb[:])
            rstd = p_l2.tile([P, 1], F32, tag="rstd2", name=f"rstd2_{q}")
            nc.vector.reciprocal(rstd[:], sd[:])
            xh2 = p_l2.tile([P, D], F16, tag="xh2", name=f"xh2_{q}")
            with nc.allow_low_precision("fp16 activations"):
                nc.vector.tensor_scalar(
                    xh2[:], x2[:, q, :], mvq[:, q, 0:1], rstd[:],
                    OP.subtract, OP.mult)
            for j in range(DC):
                ptt = ps_t2.tile([P, 8, P], F16, tag="t2", name=f"t2_{q}_{j}")
                pt = ptt[:, 0, :]
                nc.tensor.transpose(pt, xh2[:, j * P:(j + 1) * P],
                                    ident[:])
                with nc.allow_low_precision("fp16 activations"):
                    nc.vector.tensor_copy(h2T[:, j, q * P:(q + 1) * P], pt)

        # ================= Attention + proj + ReduceScatter =================
        with tc.tile_pool(name="p_e", bufs=1) as p_e, \
             tc.tile_pool(name="p_pr", bufs=3) as p_pr, \
             tc.tile_pool(name="ps_s", bufs=2, space="PSUM") as ps_s, \
             tc.tile_pool(name="ps_pv", bufs=2, space="PSUM") as ps_pv:
            for s in range(NSPAN):
                nkb = (s + 1) * (SPAN // P)
                for hp in range(2):
                    ppvs = [ps_pv.tile([DH + 1, SPAN], F32, tag=f"pv{i}",
                                       name=f"pv{hp * 2 + i}_{s}")
                            for i in range(2)]
                    es = {}

                    def scores(kb, s=s, hp=hp, es=es):
                        j = kb - s * (SPAN // P)
                        c0 = max(0, j) * P   # first valid query col
                        pst = ps_s.tile([P, 2, SPAN], F32, tag="sT",
                                        name=f"sT{hp}_{s}_{kb}")
                        for hh in range(2):
                            nc.tensor.matmul(
                                pst[:, hh, c0:SPAN],
                                kT[hh * DH:(hh + 1) * DH, hp,
                                   kb * P:(kb + 1) * P],
                                qT[hh * DH:(hh + 1) * DH, hp,
                                   s * SPAN + c0:(s + 1) * SPAN])
                        e = p_e.tile([P, 2, SPAN], F16, tag="e",
                                     bufs=SKEW + 2, name=f"e{hp}_{s}_{kb}")
                        with nc.allow_low_precision("fp16 activations"):
                            nc.scalar.activation(e[:, :, c0:SPAN],
                                                 pst[:, :, c0:SPAN],
                                                 AF.Exp, scale=0.125)
                        if j >= 0:
                            # zero the causal triangle of the diagonal block
                            nc.gpsimd.affine_select(
                                out=e[:, :, c0:c0 + P],
                                in_=e[:, :, c0:c0 + P],
                                compare_op=OP.is_ge, fill=0.0, base=0,
                                channel_multiplier=-1,
                                pattern=[[0, 2], [1, P]])
                        es[kb] = (e, c0)

                    def pv_acc(kb, s=s, hp=hp, ppvs=ppvs, nkb=nkb, es=es):
                        e, c0 = es.pop(kb)
                        for hh in range(2):
                            h = hp * 2 + hh
                            nc.tensor.matmul(
                                ppvs[hh][:, c0:SPAN],
                                vhat[:, kb, h * (DH + 1):(h + 1) * (DH + 1)],
                                e[:, hh, c0:SPAN], start=(kb == 0),
                                stop=(kb == nkb - 1))

                    for i in range(nkb + SKEW):
                        if i < nkb:
                            scores(i)
                        if i >= SKEW:
                            pv_acc(i - SKEW)
                    # normalize: att = pv / den (recip of den row, bcast)
                    for hh in range(2):
                        ppv = ppvs[hh]
                        den = p_e.tile([1, SPAN], F32, tag="den", bufs=2,
                                       name=f"den{hp}_{hh}_{s}")
                        nc.vector.tensor_copy(den[:], ppv[DH:DH + 1, :])
                        dbs = p_e.tile([DH, SPAN], F32, tag="dbs", bufs=2,
                                       name=f"dbs{hp}_{hh}_{s}")
                        nc.gpsimd.partition_broadcast(dbs[:], den[:],
                                                      channels=DH)
                        rbs = p_e.tile([DH, SPAN], F32, tag="rbs", bufs=2,
                                       name=f"rbs{hp}_{hh}_{s}")
                        nc.vector.reciprocal(rbs[:], dbs[:])
                        with nc.allow_low_precision("fp16 activations"):
                            nc.vector.tensor_tensor(
                                attT[hh * DH:(hh + 1) * DH, hp,
                                     s * SPAN:(s + 1) * SPAN],
                                ppv[0:DH, :], rbs[:], OP.mult)

                # projection partial for this token quarter, then its RS
                for mtl in range(4):
                    m = s * 4 + mtl
                    pp = ps_s.tile([P, 2, SPAN], F32, tag="sT",
                                   name=f"pr{m}")
                    for kc in range(2):
                        for n in range(2):
                            nc.tensor.matmul(
                                pp[:, n, :], attT[:, kc, m * P:(m + 1) * P],
                                wp_t[:, kc, n * SPAN:(n + 1) * SPAN],
                                start=(kc == 0), stop=(kc == 1))
                    pe = p_pr.tile([P, D], F16, tag="pe", name=f"pe{m}")
                    with nc.allow_low_precision("fp16 rs payload"):
                        nc.vector.tensor_copy(pe[:], pp[:])
                    nc.sync.dma_start(
                        rs_in[s][mtl * P:(mtl + 1) * P, :], pe[:])
                nc.gpsimd.collective_compute(
                    "ReduceScatter", OP.add, replica_groups=GROUPS,
                    ins=[rs_in[s][:]], outs=[rs_out[s][:]])

        # ================= FFN =================
        # aT reuses the dead qT/kT/vhat/attT slots (4 x 8 hid-tiles)
        aTs = [big.tile([P, HID // P // 4, TS], F16, tag=f"B{i + 1}",
                        name=f"aT{i}") for i in range(4)]

        def aT(m):
            return aTs[m // 8][:, m % 8, :]

        with tc.tile_pool(name="p_rs", bufs=2) as p_rs, \
             tc.tile_pool(name="p_l2", bufs=2) as p_l2, \
             tc.tile_pool(name="ps_t2", bufs=2, space="PSUM") as ps_t2, \
             tc.tile_pool(name="ps_f1", bufs=4, space="PSUM") as ps_f1:

            def fc1_half(ha):
                t0, t1 = ha * 256, (ha + 1) * 256
                for m in range(HID // P):
                    pft = ps_f1.tile([P, 512], F32, tag="f1",
                                     name=f"f1_{ha}_{m}")
                    pf = pft[:, 0:256]
                    for kc in range(DC):
                        nc.tensor.matmul(pf, wfc_sb[:, m, kc, :],
                                         h2T[:, kc, t0:t1],
                                         start=(kc == 0), stop=(kc == DC - 1))
                    with nc.allow_low_precision("fp16 activations"):
                        nc.scalar.activation(aT(m)[:, t0:t1], pf, AF.Gelu,
                                             bias=bfct[:, m:m + 1])

            for q in range(2):
                p5_early(q, p_rs, p_l2)
                p5_late(q, p_l2, ps_t2)
            fc1_half(0)
            for q in range(2, NQ):
                p5_early(q, p_rs, p_l2)
                p5_late(q, p_l2, ps_t2)
            fc1_half(1)

        with tc.tile_pool(name="p_w2", bufs=6) as p_w2, \
             tc.tile_pool(name="p_ot", bufs=3) as p_ot, \
             tc.tile_pool(name="ps_f2", bufs=4, space="PSUM") as ps_f2:
            p4s = [ps_f2.tile([P, 2, SPAN], F32, tag="f2", name=f"f2_{mt}")
                   for mt in range(NQ)]
            for kc in range(HID // P):
                w2 = p_w2.tile([P, D], F16, tag="w2", name=f"w2_{kc}")
                nc.sync.dma_start(w2[:], wfc2_d[kc * P:(kc + 1) * P, :])
                for mt in range(NQ):
                    for n in range(2):
                        nc.tensor.matmul(
                            p4s[mt][:, n, :],
                            aT(kc)[:, mt * P:(mt + 1) * P],
                            w2[:, n * SPAN:(n + 1) * SPAN],
                            start=(kc == 0), stop=(kc == HID // P - 1))
            for mt in range(NQ):
                for n in range(2):
                    ot = p_ot.tile([P, SPAN], F32, tag="ot",
                                   name=f"ot{n}_{mt}")
                    nc.vector.tensor_tensor(
                        ot[:], p4s[mt][:, n, :],
                        x2[:, mt, n * SPAN:(n + 1) * SPAN], OP.add)
                    nc.vector.tensor_tensor(
                        ot[:], ot[:],
                        bfc2b[:, n * SPAN:(n + 1) * SPAN], OP.add)
                    nc.sync.dma_start(
                        out_d[mt * P:(mt + 1) * P, n * SPAN:(n + 1) * SPAN],
                        ot[:])

        big_cm.__exit__(None, None, None)
        cst_cm.__exit__(None, None, None)

    nc.finalize()
    return nc


def shard_inputs(inputs):
    """Full inputs -> per-core in_maps (8 cores)."""
    f = lambda a: np.ascontiguousarray(np.asarray(a, dtype=np.float32))
    h = lambda a: np.ascontiguousarray(np.asarray(a, dtype=np.float16))
    x = f(inputs["x"])
    w_attn, b_attn = f(inputs["w_attn"]), f(inputs["b_attn"])
    w_proj, b_proj = f(inputs["w_proj"]), f(inputs["b_proj"])
    ln1_g, ln1_b = f(inputs["ln1_g"]), f(inputs["ln1_b"])
    ln2_g, ln2_b = f(inputs["ln2_g"]), f(inputs["ln2_b"])
    w_fc, b_fc = f(inputs["w_fc"]), f(inputs["b_fc"])
    w_fc2, b_fc2 = f(inputs["w_fc2"]), f(inputs["b_fc2"])

    # fold LN1 gamma into W_attn rows, beta into the qkv bias
    # LN(x)@W = xhat@(g*W) + b@W  (b uses the unscaled W)
    w_attn_f = w_attn * ln1_g[:, None]
    b_attn_f = b_attn + ln1_b @ w_attn
    # fold LN2 gamma into W_fc rows, beta into b_fc
    w_fc_f = w_fc * ln2_g[:, None]
    b_fc_f = b_fc + ln2_b @ w_fc

    wfc_r = h(w_fc_f.reshape(DC, P, HID // P, P).transpose(2, 1, 0, 3))
    bfct = np.ascontiguousarray(b_fc_f.reshape(HID // P, P).T)
    bfc2b = np.ascontiguousarray(np.broadcast_to(b_fc2.reshape(1, D), (P, D)))
    wfc2_h = h(w_fc2)

    in_maps = []
    for c in range(8):
        g, tp = c // 4, c % 4
        sl = slice(tp * FQ, (tp + 1) * FQ)
        bq = b_attn_f[0 * D:1 * D][sl]
        bk = b_attn_f[1 * D:2 * D][sl]
        bv = b_attn_f[2 * D:3 * D][sl]
        bqk = np.concatenate([bq.reshape(2, P).T, bk.reshape(2, P).T], axis=1)
        bvb = np.broadcast_to(bv.reshape(1, FQ), (P, FQ))
        xs = np.concatenate(
            [x[g, q * SPAN + tp * P: q * SPAN + (tp + 1) * P]
             for q in range(NQ)], axis=0) + b_proj.reshape(1, D)
        xt = np.ascontiguousarray(x[g].T.reshape(DC, P, T))
        in_maps.append({
            "xt": h(xt),
            "xs": h(xs),
            "wq": h(w_attn_f[:, 0 * D:1 * D][:, sl]),
            "wk": h(w_attn_f[:, 1 * D:2 * D][:, sl]),
            "wv": h(w_attn_f[:, 2 * D:3 * D][:, sl]),
            "bqk": np.ascontiguousarray(bqk),
            "bvb": np.ascontiguousarray(bvb),
            "wp": h(w_proj[sl, :]),
            "wfc": wfc_r, "bfc": bfct,
            "wfc2": wfc2_h, "bfc2b": bfc2b,
        })
    return in_maps


def assemble(results):
    out = np.empty((2, T, D), dtype=np.float32)
    for c in range(8):
        g, tp = c // 4, c % 4
        r = np.asarray(results[c]["out"])
        for q in range(NQ):
            out[g, q * SPAN + tp * P: q * SPAN + (tp + 1) * P] = \
                r[q * P:(q + 1) * P]
    return out


_NC = None


def kernel(**inputs):
    global _NC
    if _NC is None:
        _NC = build_nc()
    in_maps = shard_inputs(inputs)
    res = run_bass_kernel_spmd(_NC, in_maps, list(range(8)))
    return assemble(res.results)
